# revision 6
# baseline (speedup 1.0000x reference)
"""SAGAN-style self-attention block on 8 TRN2 NeuronCores.

Data-parallel over batch (B=8): core i processes sample i with replicated
(tiny) conv weights. No collectives.

Per-core math (pix = 64*64 = 4096, C=256, M = 32*32 = 1024 pooled keys):
  g = x @ Wg                      [4096, 32]
  f = maxpool2x2(x @ Wf)          [1024, 32]
  h = maxpool2x2(x @ Wh)          [1024, 128]
  s = g @ f.T                     [4096, 1024]
  beta = softmax(s, -1)
  o = beta @ h                    [4096, 128]
  out = gamma * (o @ Wo) + x      [4096, 256]

Device layout trick: everything runs transposed (channels/keys on SBUF
partitions) so all matmuls contract on the partition axis.  The host
pre-transposes x to [256, 4096] per sample and transposes the output back.
gamma is folded into Wo on the host; biases are all-zero by problem spec.
Softmax is computed without max-subtraction (logits are O(30), safe in f32)
and normalization is deferred: e = exp(s), r[n] = sum_m e[m,n] via a
ones-vector matmul, o_unnorm = e @ h, and the per-pixel 1/r (broadcast to all
partitions with a K=1 ones matmul) is applied after.
"""

import numpy as np

import concourse.bass as bass
import concourse.mybir as mybir
from concourse import bacc
import concourse.tile as tile
from concourse.bass_utils import run_bass_kernel_spmd
from concourse.masks import make_identity

F32 = mybir.dt.float32
F32R = mybir.dt.float32r
BF16 = mybir.dt.bfloat16

P = 128
NPIX = 4096          # 64*64 pixels
NCHUNK = 8           # pixel chunks of 512
PIX = NPIX // NCHUNK  # 512
M = 1024             # pooled keys
MCH = 8              # m chunks of 128
C = 256              # channels (2 k-chunks of 128)
C8 = 32              # C//8
C2 = 128             # C//2

_CACHED = {}


def _build():
    nc = bacc.Bacc()

    xt_ext = nc.declare_dram_parameter("xt", [C, NPIX], F32, isOutput=False)
    wf_ext = nc.declare_dram_parameter("Wf", [C, C8], F32, isOutput=False)
    wg_ext = nc.declare_dram_parameter("Wg", [C, C8], F32, isOutput=False)
    wh_ext = nc.declare_dram_parameter("Wh", [C, C2], F32, isOutput=False)
    wo_ext = nc.declare_dram_parameter("Wo", [C2, C], F32, isOutput=False)
    out_ext = nc.declare_dram_parameter("out", [C, NPIX], F32, isOutput=True)

    xt_r = xt_ext.rearrange("(ko p) n -> p ko n", p=P)
    out_r = out_ext.rearrange("(j p) n -> p j n", p=P)

    with tile.TileContext(nc) as tc:
        with (
            tc.tile_pool(name="const", bufs=1) as constp,
            tc.tile_pool(name="big", bufs=1) as bigp,
            tc.tile_pool(name="stage", bufs=3) as stagep,
            tc.tile_pool(name="outp", bufs=3) as outp,
            tc.tile_pool(name="ps1", bufs=3, space="PSUM") as ps1,
            tc.tile_pool(name="pss", bufs=2, space="PSUM") as pss,
            tc.tile_pool(name="psr", bufs=1, space="PSUM") as psr,
        ):
            # ---- constants / weights -------------------------------------
            wg_sb = constp.tile([P, 2, C8], F32)
            nc.sync.dma_start(out=wg_sb, in_=wg_ext.rearrange("(ko p) m -> p ko m", p=P))
            wf_sb = constp.tile([P, 2, C8], F32)
            nc.sync.dma_start(out=wf_sb, in_=wf_ext.rearrange("(ko p) m -> p ko m", p=P))
            wh_sb = constp.tile([P, 2, C2], F32)
            nc.sync.dma_start(out=wh_sb, in_=wh_ext.rearrange("(ko p) m -> p ko m", p=P))
            wo_f32 = constp.tile([C2, C], F32)
            nc.sync.dma_start(out=wo_f32, in_=wo_ext[:])
            wo_sb = constp.tile([C2, 2, P], BF16)
            nc.vector.tensor_copy(out=wo_sb, in_=wo_f32.rearrange("k (j m) -> k j m", j=2))
            wg_bf = constp.tile([P, 2, C8], BF16)
            nc.vector.tensor_copy(out=wg_bf, in_=wg_sb)
            wf_bf = constp.tile([P, 2, C8], BF16)
            nc.vector.tensor_copy(out=wf_bf, in_=wf_sb)
            wh_bf = constp.tile([P, 2, C2], BF16)
            nc.vector.tensor_copy(out=wh_bf, in_=wh_sb)

            ident = constp.tile([P, P], BF16)
            make_identity(nc, ident)
            ones_m = constp.tile([P, 1], BF16)
            nc.vector.memset(ones_m, 1.0)
            ones_row = constp.tile([1, P], BF16)
            nc.vector.memset(ones_row, 1.0)

            # ---- big persistent activations ------------------------------
            xt_sb = bigp.tile([P, 2, NPIX], F32)          # 32 KB/part
            xb_sb = bigp.tile([P, 2, NPIX], BF16)         # bf16 copy for convs
            gt_sb = bigp.tile([C8, NPIX], BF16)           # g^T
            ft_sb = bigp.tile([C8, M], BF16)              # f^T pooled
            ht_sb = bigp.tile([C2, M], BF16)              # h^T pooled [c', m]
            h_sb = bigp.tile([P, MCH, C2], BF16)          # h [m, c'] per m-chunk
            et_sb = bigp.tile([P, MCH, NPIX], BF16)       # e^T = exp(s^T), 64 KB/part
            ot_sb = bigp.tile([C2, NPIX], BF16)           # o^T unnorm-scaled
            r_row = bigp.tile([1, NPIX], F32)             # softmax denominators
            recip = bigp.tile([1, NPIX], BF16)            # 1/r
            scale_rep = bigp.tile([P, NPIX], F32)         # 1/r replicated to all parts

            def ns(n):
                return slice(n * PIX, (n + 1) * PIX)

            # ---- load x, convs, pool pass1 -------------------------------
            for n in range(NCHUNK):
                nc.sync.dma_start(out=xt_sb[:, :, ns(n)], in_=xt_r[:, :, ns(n)])
                nc.vector.tensor_copy(out=xb_sb[:, :, ns(n)], in_=xt_sb[:, :, ns(n)])
                xr = xb_sb[:, :, ns(n)]
                # conv g -> gt
                pg = ps1.tile([P, PIX], F32, tag="pb")
                for ko in range(2):
                    nc.tensor.matmul(pg[:C8], lhsT=wg_bf[:, ko],
                                     rhs=xr[:, ko], start=(ko == 0), stop=(ko == 1))
                nc.vector.tensor_copy(out=gt_sb[:, ns(n)], in_=pg[:C8])
                # conv f -> 2x2 maxpool via one strided XY reduce
                pf = ps1.tile([P, PIX], F32, tag="pb")
                for ko in range(2):
                    nc.tensor.matmul(pf[:C8], lhsT=wf_bf[:, ko],
                                     rhs=xr[:, ko], start=(ko == 0), stop=(ko == 1))
                pfv = pf[:C8].rearrange("p (r a c b) -> p r c a b", r=4, a=2, b=2)
                nc.vector.tensor_reduce(out=ft_sb[:, n * 128:(n + 1) * 128], in_=pfv,
                                        axis=mybir.AxisListType.XY, op=mybir.AluOpType.max)
                # conv h -> 2x2 maxpool
                ph = ps1.tile([P, PIX], F32, tag="pb")
                for ko in range(2):
                    nc.tensor.matmul(ph, lhsT=wh_bf[:, ko],
                                     rhs=xr[:, ko], start=(ko == 0), stop=(ko == 1))
                phv = ph.rearrange("p (r a c b) -> p r c a b", r=4, a=2, b=2)
                nc.vector.tensor_reduce(out=ht_sb[:, n * 128:(n + 1) * 128], in_=phv,
                                        axis=mybir.AxisListType.XY, op=mybir.AluOpType.max)

            # ---- transpose h to [m, c'] ----------------------------------
            for mi in range(MCH):
                pt = ps1.tile([P, P], BF16, tag="pb")
                nc.tensor.transpose(pt, ht_sb[:, mi * P:(mi + 1) * P], ident)
                nc.vector.tensor_copy(out=h_sb[:, mi], in_=pt)

            # ---- attention: s, exp, r, recip, bcast, o, final ------------
            for n in range(NCHUNK):
                # s^T and exp, two m-chunks per psum tile
                for half in range(4):
                    ps_t = pss.tile([P, 2, PIX], F32, tag="s")
                    for q in range(2):
                        mi = 2 * half + q
                        nc.tensor.matmul(ps_t[:, q],
                                         lhsT=ft_sb[:C8, mi * P:(mi + 1) * P],
                                         rhs=gt_sb[:, ns(n)],
                                         start=True, stop=True)
                    nc.scalar.activation(out=et_sb[:, 2 * half:2 * half + 2, ns(n)],
                                         in_=ps_t, func=mybir.ActivationFunctionType.Exp)
                # r[n-chunk] = sum_m e
                pr = psr.tile([1, PIX], F32, tag="r")
                for mi in range(MCH):
                    nc.tensor.matmul(pr, lhsT=ones_m, rhs=et_sb[:, mi, ns(n)],
                                     start=(mi == 0), stop=(mi == MCH - 1))
                nc.vector.tensor_copy(out=r_row[:, ns(n)], in_=pr)
                with nc.allow_low_precision(reason="1/r in bf16: r only normalizes softmax; 0.4% is fine"):
                    nc.vector.reciprocal(out=recip[:, ns(n)], in_=r_row[:, ns(n)])
                # broadcast 1/r to all partitions via K=1 ones matmul
                pb = ps1.tile([P, PIX], F32, tag="pb")
                nc.tensor.matmul(pb, lhsT=ones_row, rhs=recip[:, ns(n)],
                                 start=True, stop=True)
                nc.vector.tensor_copy(out=scale_rep[:, ns(n)], in_=pb)
                # o = e @ h (unnormalized), then scale by 1/r
                po = ps1.tile([P, PIX], F32, tag="pb")
                for mi in range(MCH):
                    nc.tensor.matmul(po, lhsT=h_sb[:, mi], rhs=et_sb[:, mi, ns(n)],
                                     start=(mi == 0), stop=(mi == MCH - 1))
                nc.vector.tensor_tensor(out=ot_sb[:, ns(n)], in0=po,
                                        in1=scale_rep[:, ns(n)], op=mybir.AluOpType.mult)
                # final conv (gamma pre-folded into Wo) + residual, transposed
                for j in range(2):
                    pf2 = ps1.tile([P, PIX], F32, tag="pb")
                    nc.tensor.matmul(pf2, lhsT=wo_sb[:, j], rhs=ot_sb[:, ns(n)],
                                     start=True, stop=True)
                    ob = outp.tile([P, PIX], F32)
                    nc.vector.tensor_tensor(out=ob, in0=pf2, in1=xt_sb[:, j, ns(n)],
                                            op=mybir.AluOpType.add)
                    nc.sync.dma_start(out=out_r[:, j, ns(n)], in_=ob)

    nc.finalize()
    return nc


def _get_nc():
    if "nc" not in _CACHED:
        _CACHED["nc"] = _build()
    return _CACHED["nc"]


def _make_in_maps(inputs):
    x = np.asarray(inputs["x"], dtype=np.float32)        # [8, 64, 64, 256]
    B = x.shape[0]
    for bname in ("bf", "bg", "bh", "bo"):
        b = np.asarray(inputs[bname])
        assert np.max(np.abs(b)) == 0.0, f"{bname} must be zero (spec fill=zeros)"
    gamma = float(np.asarray(inputs["gamma"]).reshape(-1)[0])
    wo_eff = (np.asarray(inputs["Wo"], dtype=np.float32) * gamma).astype(np.float32)
    wf = np.ascontiguousarray(np.asarray(inputs["Wf"], dtype=np.float32))
    wg = np.ascontiguousarray(np.asarray(inputs["Wg"], dtype=np.float32))
    wh = np.ascontiguousarray(np.asarray(inputs["Wh"], dtype=np.float32))
    in_maps = []
    for i in range(B):
        xt = np.ascontiguousarray(x[i].reshape(NPIX, C).T)   # [256, 4096]
        in_maps.append({"xt": xt, "Wf": wf, "Wg": wg, "Wh": wh, "Wo": wo_eff})
    return in_maps


def _gather(results):
    outs = []
    for r in results:
        ot = np.asarray(r["out"])          # [256, 4096]
        outs.append(ot.T.reshape(64, 64, C))
    return np.stack(outs).astype(np.float32)


def kernel(**inputs):
    nc = _get_nc()
    in_maps = _make_in_maps(inputs)
    res = run_bass_kernel_spmd(nc, in_maps, core_ids=list(range(len(in_maps))))
    return _gather(res.results)


def bench(inputs, trace=True):
    """Run with profiling; returns (output, BassKernelResults)."""
    nc = _get_nc()
    in_maps = _make_in_maps(inputs)
    res = run_bass_kernel_spmd(nc, in_maps, core_ids=list(range(len(in_maps))),
                               trace=trace)
    return _gather(res.results), res


# revision 7
# speedup vs baseline: 1.0935x; 1.0935x over previous
"""SAGAN-style self-attention block on 8 TRN2 NeuronCores.

Data-parallel over batch (B=8): core i processes sample i with replicated
(tiny) conv weights. No collectives.

Per-core math (pix = 64*64 = 4096, C=256, M = 32*32 = 1024 pooled keys):
  g = x @ Wg                      [4096, 32]
  f = maxpool2x2(x @ Wf)          [1024, 32]
  h = maxpool2x2(x @ Wh)          [1024, 128]
  s = g @ f.T                     [4096, 1024]
  beta = softmax(s, -1)
  o = beta @ h                    [4096, 128]
  out = gamma * (o @ Wo) + x      [4096, 256]

Device layout trick: everything runs transposed (channels/keys on SBUF
partitions) so all matmuls contract on the partition axis.  The host
pre-transposes x to [256, 4096] per sample and transposes the output back.
gamma is folded into Wo on the host; biases are all-zero by problem spec.
Softmax is computed without max-subtraction (logits are O(30), safe in f32)
and normalization is deferred: e = exp(s), r[n] = sum_m e[m,n] via a
ones-vector matmul, o_unnorm = e @ h, and the per-pixel 1/r (broadcast to all
partitions with a K=1 ones matmul) is applied after.
"""

import numpy as np

import concourse.bass as bass
import concourse.mybir as mybir
from concourse import bacc
import concourse.tile as tile
from concourse.bass_utils import run_bass_kernel_spmd
from concourse.masks import make_identity

F32 = mybir.dt.float32
F32R = mybir.dt.float32r
BF16 = mybir.dt.bfloat16

P = 128
NPIX = 4096          # 64*64 pixels
NCHUNK = 8           # pixel chunks of 512
PIX = NPIX // NCHUNK  # 512
M = 1024             # pooled keys
MCH = 8              # m chunks of 128
C = 256              # channels (2 k-chunks of 128)
C8 = 32              # C//8
C2 = 128             # C//2

_CACHED = {}


def _build():
    nc = bacc.Bacc()

    xt_ext = nc.declare_dram_parameter("xt", [C, NPIX], F32, isOutput=False)
    wf_ext = nc.declare_dram_parameter("Wf", [C, C8], F32, isOutput=False)
    wg_ext = nc.declare_dram_parameter("Wg", [C, C8], F32, isOutput=False)
    wh_ext = nc.declare_dram_parameter("Wh", [C, C2], F32, isOutput=False)
    wo_ext = nc.declare_dram_parameter("Wo", [C2, C], F32, isOutput=False)
    out_ext = nc.declare_dram_parameter("out", [C, NPIX], F32, isOutput=True)

    xt_r = xt_ext.rearrange("(ko p) n -> p ko n", p=P)
    out_r = out_ext.rearrange("(j p) n -> p j n", p=P)

    with tile.TileContext(nc) as tc:
        with (
            tc.tile_pool(name="const", bufs=1) as constp,
            tc.tile_pool(name="big", bufs=1) as bigp,
            tc.tile_pool(name="stage", bufs=3) as stagep,
            tc.tile_pool(name="outp", bufs=3) as outp,
            tc.tile_pool(name="ps1", bufs=3, space="PSUM") as ps1,
            tc.tile_pool(name="pss", bufs=2, space="PSUM") as pss,
            tc.tile_pool(name="psr", bufs=1, space="PSUM") as psr,
        ):
            # ---- constants / weights -------------------------------------
            wg_sb = constp.tile([P, 2, C8], F32)
            nc.sync.dma_start(out=wg_sb, in_=wg_ext.rearrange("(ko p) m -> p ko m", p=P))
            wf_sb = constp.tile([P, 2, C8], F32)
            nc.sync.dma_start(out=wf_sb, in_=wf_ext.rearrange("(ko p) m -> p ko m", p=P))
            wh_sb = constp.tile([P, 2, C2], F32)
            nc.sync.dma_start(out=wh_sb, in_=wh_ext.rearrange("(ko p) m -> p ko m", p=P))
            wo_f32 = constp.tile([C2, C], F32)
            nc.sync.dma_start(out=wo_f32, in_=wo_ext[:])
            wo_sb = constp.tile([C2, 2, P], BF16)
            nc.vector.tensor_copy(out=wo_sb, in_=wo_f32.rearrange("k (j m) -> k j m", j=2))
            wg_bf = constp.tile([P, 2, C8], BF16)
            nc.vector.tensor_copy(out=wg_bf, in_=wg_sb)
            wf_bf = constp.tile([P, 2, C8], BF16)
            nc.vector.tensor_copy(out=wf_bf, in_=wf_sb)
            wh_bf = constp.tile([P, 2, C2], BF16)
            nc.vector.tensor_copy(out=wh_bf, in_=wh_sb)

            ident = constp.tile([P, P], BF16)
            make_identity(nc, ident)
            ones_m = constp.tile([P, 1], BF16)
            nc.vector.memset(ones_m, 1.0)
            ones_row = constp.tile([1, P], BF16)
            nc.vector.memset(ones_row, 1.0)

            # ---- big persistent activations ------------------------------
            xt_sb = bigp.tile([P, 2, NPIX], F32)          # 32 KB/part
            xb_sb = bigp.tile([P, 2, NPIX], BF16)         # bf16 copy for convs
            gt_sb = bigp.tile([C8, NPIX], BF16)           # g^T
            ft_sb = bigp.tile([C8, M], BF16)              # f^T pooled
            ht_sb = bigp.tile([C2, M], BF16)              # h^T pooled [c', m]
            h_sb = bigp.tile([P, MCH, C2], BF16)          # h [m, c'] per m-chunk
            et_sb = bigp.tile([P, MCH, NPIX], BF16)       # e^T = exp(s^T), 64 KB/part
            ot_sb = bigp.tile([C2, NPIX], BF16)           # o^T unnorm-scaled
            r_bf = bigp.tile([1, NPIX], BF16)             # softmax denominators (bf16)
            scale_rep = bigp.tile([P, NPIX], F32)         # 1/r replicated to all parts

            def ns(n):
                return slice(n * PIX, (n + 1) * PIX)

            # ---- load x, convs, pool pass1 -------------------------------
            for n in range(NCHUNK):
                nc.sync.dma_start(out=xt_sb[:, :, ns(n)], in_=xt_r[:, :, ns(n)])
                nc.vector.tensor_copy(out=xb_sb[:, :, ns(n)], in_=xt_sb[:, :, ns(n)])
                xr = xb_sb[:, :, ns(n)]
                # conv g -> gt
                pg = ps1.tile([P, PIX], F32, tag="pb")
                for ko in range(2):
                    nc.tensor.matmul(pg[:C8], lhsT=wg_bf[:, ko],
                                     rhs=xr[:, ko], start=(ko == 0), stop=(ko == 1))
                nc.vector.tensor_copy(out=gt_sb[:, ns(n)], in_=pg[:C8])
                # conv f -> 2x2 maxpool via one strided XY reduce
                pf = ps1.tile([P, PIX], F32, tag="pb")
                for ko in range(2):
                    nc.tensor.matmul(pf[:C8], lhsT=wf_bf[:, ko],
                                     rhs=xr[:, ko], start=(ko == 0), stop=(ko == 1))
                pfv = pf[:C8].rearrange("p (r a c b) -> p r c a b", r=4, a=2, b=2)
                nc.vector.tensor_reduce(out=ft_sb[:, n * 128:(n + 1) * 128], in_=pfv,
                                        axis=mybir.AxisListType.XY, op=mybir.AluOpType.max)
                # conv h -> 2x2 maxpool
                ph = ps1.tile([P, PIX], F32, tag="pb")
                for ko in range(2):
                    nc.tensor.matmul(ph, lhsT=wh_bf[:, ko],
                                     rhs=xr[:, ko], start=(ko == 0), stop=(ko == 1))
                phv = ph.rearrange("p (r a c b) -> p r c a b", r=4, a=2, b=2)
                nc.vector.tensor_reduce(out=ht_sb[:, n * 128:(n + 1) * 128], in_=phv,
                                        axis=mybir.AxisListType.XY, op=mybir.AluOpType.max)

            # ---- transpose h to [m, c'] ----------------------------------
            for mi in range(MCH):
                pt = ps1.tile([P, P], BF16, tag="pb")
                nc.tensor.transpose(pt, ht_sb[:, mi * P:(mi + 1) * P], ident)
                nc.vector.tensor_copy(out=h_sb[:, mi], in_=pt)

            # ---- attention: s, exp, r, recip, bcast, o, final ------------
            for n in range(NCHUNK):
                # s^T and exp, two m-chunks per psum tile
                for half in range(4):
                    ps_t = pss.tile([P, 2, PIX], F32, tag="s")
                    for q in range(2):
                        mi = 2 * half + q
                        nc.tensor.matmul(ps_t[:, q],
                                         lhsT=ft_sb[:C8, mi * P:(mi + 1) * P],
                                         rhs=gt_sb[:, ns(n)],
                                         start=True, stop=True)
                    nc.scalar.activation(out=et_sb[:, 2 * half:2 * half + 2, ns(n)],
                                         in_=ps_t, func=mybir.ActivationFunctionType.Exp)
                # r[n-chunk] = sum_m e
                pr = psr.tile([1, PIX], F32, tag="r")
                for mi in range(MCH):
                    nc.tensor.matmul(pr, lhsT=ones_m, rhs=et_sb[:, mi, ns(n)],
                                     start=(mi == 0), stop=(mi == MCH - 1))
                nc.vector.tensor_copy(out=r_bf[:, ns(n)], in_=pr)
                # broadcast r to all partitions via K=1 ones matmul, then
                # 1/r = exp(-ln(r)) on ACT (both fns live in one table set)
                pb = ps1.tile([P, PIX], F32, tag="pb")
                nc.tensor.matmul(pb, lhsT=ones_row, rhs=r_bf[:, ns(n)],
                                 start=True, stop=True)
                ln_t = stagep.tile([P, PIX], F32, tag="ln")
                nc.scalar.activation(out=ln_t, in_=pb,
                                     func=mybir.ActivationFunctionType.Ln)
                nc.scalar.activation(out=scale_rep[:, ns(n)], in_=ln_t,
                                     func=mybir.ActivationFunctionType.Exp, scale=-1.0)
                # o = e @ h (unnormalized), then scale by 1/r
                po = ps1.tile([P, PIX], F32, tag="pb")
                for mi in range(MCH):
                    nc.tensor.matmul(po, lhsT=h_sb[:, mi], rhs=et_sb[:, mi, ns(n)],
                                     start=(mi == 0), stop=(mi == MCH - 1))
                nc.vector.tensor_tensor(out=ot_sb[:, ns(n)], in0=po,
                                        in1=scale_rep[:, ns(n)], op=mybir.AluOpType.mult)
                # final conv (gamma pre-folded into Wo) + residual, transposed
                for j in range(2):
                    pf2 = ps1.tile([P, PIX], F32, tag="pb")
                    nc.tensor.matmul(pf2, lhsT=wo_sb[:, j], rhs=ot_sb[:, ns(n)],
                                     start=True, stop=True)
                    ob = outp.tile([P, PIX], F32)
                    nc.vector.tensor_tensor(out=ob, in0=pf2, in1=xt_sb[:, j, ns(n)],
                                            op=mybir.AluOpType.add)
                    nc.sync.dma_start(out=out_r[:, j, ns(n)], in_=ob)

    nc.finalize()
    return nc


def _get_nc():
    if "nc" not in _CACHED:
        _CACHED["nc"] = _build()
    return _CACHED["nc"]


def _make_in_maps(inputs):
    x = np.asarray(inputs["x"], dtype=np.float32)        # [8, 64, 64, 256]
    B = x.shape[0]
    for bname in ("bf", "bg", "bh", "bo"):
        b = np.asarray(inputs[bname])
        assert np.max(np.abs(b)) == 0.0, f"{bname} must be zero (spec fill=zeros)"
    gamma = float(np.asarray(inputs["gamma"]).reshape(-1)[0])
    wo_eff = (np.asarray(inputs["Wo"], dtype=np.float32) * gamma).astype(np.float32)
    wf = np.ascontiguousarray(np.asarray(inputs["Wf"], dtype=np.float32))
    wg = np.ascontiguousarray(np.asarray(inputs["Wg"], dtype=np.float32))
    wh = np.ascontiguousarray(np.asarray(inputs["Wh"], dtype=np.float32))
    in_maps = []
    for i in range(B):
        xt = np.ascontiguousarray(x[i].reshape(NPIX, C).T)   # [256, 4096]
        in_maps.append({"xt": xt, "Wf": wf, "Wg": wg, "Wh": wh, "Wo": wo_eff})
    return in_maps


def _gather(results):
    outs = []
    for r in results:
        ot = np.asarray(r["out"])          # [256, 4096]
        outs.append(ot.T.reshape(64, 64, C))
    return np.stack(outs).astype(np.float32)


def kernel(**inputs):
    nc = _get_nc()
    in_maps = _make_in_maps(inputs)
    res = run_bass_kernel_spmd(nc, in_maps, core_ids=list(range(len(in_maps))))
    return _gather(res.results)


def bench(inputs, trace=True):
    """Run with profiling; returns (output, BassKernelResults)."""
    nc = _get_nc()
    in_maps = _make_in_maps(inputs)
    res = run_bass_kernel_spmd(nc, in_maps, core_ids=list(range(len(in_maps))),
                               trace=trace)
    return _gather(res.results), res
